# revision 4
# baseline (speedup 1.0000x reference)
"""Trainium2 Bass kernel v2 for the 2-layer Chebyshev GCN (K=3).

Contract: kernel(**inputs) takes FULL unsharded inputs, returns FULL output.

Design (8 cores, SPMD single NEFF), optimized for the observed cost law of
this execution stack (cost ~ dependency-graph depth, ~5ms/level + base):

  - Nodes sharded contiguously; within each core rows are PERMUTED by
    descending weighted degree (host-side) so rows of similar degree share
    128-row blocks. Output is un-permuted on the host.
  - Edges are packed DEST-ALIGNED: slot (partition p, column CHOFF[b]+j)
    holds the j-th incident edge of the row at (block b, partition p).
    Blocks are grouped (host-chosen) with a uniform per-group column count
    K_g, so a WHOLE GROUP is one rectangular [128, nblk*K_g] slot array.
  - propagate = per group: ONE multi-index indirect gather (table rows for
    every slot), ONE broadcast multiply by slot weights, ONE strided reduce
    over the slot axis -> segment sums for all rows of the group.
    Weights are pre-scaled on host: w' = -dis[row] * w; the gather table
    holds dis * T, so the reduce directly yields (L_hat T)[row].
  - Dense phases run TRANSPOSED (feature-major): out^T = sum_k W_k^T @ T_k^T
    with K=feature contraction, chunked over nodes. Feature-major operands
    come from DRAM round-trip transposed reads (DMA with transposed access
    pattern), never PE transposes. BatchNorm stats are free-axis reduces in
    feature-major layout; BN apply is one fused tensor_scalar.
  - Cross-core redistribution of tables is an AllGather; BN stats use one
    AllReduce.
"""

import math
import sys

import numpy as np

sys.path.insert(0, "/opt/trn_rl_repo")

import ml_dtypes

BF16 = ml_dtypes.bfloat16

N_F32 = np.float32


class Meta:
    pass


# ---------------------------------------------------------------------------
# Host-side preprocessing
# ---------------------------------------------------------------------------


def _host_prep(x, edge_index, edge_weight, W1, b1, W2, b2, bn_gamma, bn_beta,
               lin_W, lin_b, n_cores=8, group_budget_bytes=20 * 1024):
    m = Meta()
    N, in_f = x.shape
    E = edge_index.shape[1]
    m.N, m.E, m.C = int(N), int(E), int(n_cores)
    m.in_f = int(in_f)
    m.c1 = int(W1.shape[2])
    m.c2 = int(W2.shape[2])
    m.out_f = int(lin_W.shape[0])
    assert N % n_cores == 0
    m.RPC = N // n_cores
    m.NB = (m.RPC + 127) // 128
    m.NP = m.NB * 128
    m.TN = m.C * m.NP
    m.F = max(m.in_f, m.c1, m.c2)

    row = np.asarray(edge_index[0], dtype=np.int64)
    col = np.asarray(edge_index[1], dtype=np.int64)
    w = np.asarray(edge_weight, dtype=np.float64)

    deg = np.bincount(row, weights=w, minlength=N)
    dis = np.where(deg > 0, 1.0 / np.sqrt(np.maximum(deg, 1e-300)), 0.0)

    # per-core degree-descending permutation: pos[n] in [0, RPC)
    pos = np.empty(N, dtype=np.int64)
    for c in range(m.C):
        lo = c * m.RPC
        order = np.argsort(-deg[lo:lo + m.RPC], kind="stable")
        pos[lo + order] = np.arange(m.RPC)
    m.pos = pos
    gpos = (np.arange(N) // m.RPC) * m.NP + pos          # table coordinate

    owner = row // m.RPC
    p_local = pos[row]
    part = p_local % 128
    blk = p_local // 128

    rid = owner * m.NP + p_local
    rcount = np.bincount(rid, minlength=m.C * m.NP)
    # per-(core, block) max row count -> per-block K, maxed across cores
    kb = rcount.reshape(m.C, m.NB, 128).max(axis=2).max(axis=0)  # [NB]
    kb = np.maximum(kb, 1)

    # group blocks: uniform K_g = max kb in group (kb is descending-ish),
    # nblk_g * K_g * F * 2B <= group_budget_bytes
    kcols_budget = group_budget_bytes // (m.F * 2)
    m.GW = kcols_budget * m.F                # gather tile elems per partition
    groups = []          # list of (b0, nblk, Kg)
    b0 = 0
    while b0 < m.NB:
        Kg = int(max(kb[b0], 1))
        nblk = max(1, min(m.NB - b0, kcols_budget // Kg))
        Kg = int(kb[b0:b0 + nblk].max())
        # re-shrink if the true max makes it overflow
        nblk = max(1, min(nblk, kcols_budget // Kg))
        Kg = int(kb[b0:b0 + nblk].max())
        groups.append((b0, nblk, Kg))
        b0 += nblk
    m.groups = groups
    goff = [0]
    for (_, nblk, Kg) in groups:
        goff.append(goff[-1] + nblk * Kg)
    m.GOFF = goff
    m.CH = goff[-1]

    # per-block column offset within the group-padded layout
    choff = np.zeros(m.NB, dtype=np.int64)
    kg_of_block = np.zeros(m.NB, dtype=np.int64)
    for gi, (gb0, nblk, Kg) in enumerate(groups):
        for bi in range(nblk):
            choff[gb0 + bi] = goff[gi] + bi * Kg
            kg_of_block[gb0 + bi] = Kg

    # within-row slot index for every edge
    order = np.argsort(rid, kind="stable")
    rstart = np.concatenate(([0], np.cumsum(rcount)))[:-1]
    within = np.empty(E, dtype=np.int64)
    within[order] = np.arange(E) - rstart[rid[order]]

    slot_col = choff[blk] + within                        # column in [0, CH)
    wprime = (-dis[row] * w).astype(N_F32)
    tcol = gpos[col].astype(np.int32)

    in_maps = []
    shared = _shared_consts(m, W1, b1, W2, b2, bn_gamma, bn_beta, lin_W,
                            lin_b)
    for c in range(m.C):
        sel = owner == c
        col_arr = np.zeros((128, m.CH), dtype=np.int32)
        w_arr = np.zeros((128, m.CH), dtype=N_F32)
        col_arr[part[sel], slot_col[sel]] = tcol[sel]
        w_arr[part[sel], slot_col[sel]] = wprime[sel]

        lo = c * m.RPC
        myorder = np.argsort(pos[lo:lo + m.RPC], kind="stable")  # pos->node
        xp = np.zeros((m.NP, m.in_f), dtype=N_F32)
        xp[:m.RPC] = np.asarray(x[lo:lo + m.RPC], dtype=N_F32)[myorder]
        disl = np.zeros((128, m.NB), dtype=N_F32)
        dl = dis[lo:lo + m.RPC][myorder]                  # by position
        disl[np.arange(m.RPC) % 128, np.arange(m.RPC) // 128] = dl

        im = dict(shared)
        im["xs"] = xp.astype(BF16)
        im["colsb"] = col_arr
        im["wsb"] = w_arr.astype(BF16)
        im["disl"] = disl
        in_maps.append(im)
    return m, in_maps


def _shared_consts(m, W1, b1, W2, b2, bn_gamma, bn_beta, lin_W, lin_b):
    W1 = np.asarray(W1, N_F32)
    W2 = np.asarray(W2, N_F32)
    sh = {}
    for k in range(3):
        sh[f"w1_{k}"] = W1[k].astype(BF16)                # [in_f, c1]
        sh[f"w2_{k}"] = W2[k].astype(BF16)                # [c1, c2]
    sh["linwt"] = np.ascontiguousarray(
        np.asarray(lin_W, N_F32).T).astype(BF16)          # [c2, out_f]
    sh["b1c"] = np.asarray(b1, N_F32)[:, None].copy()     # [c1, 1]
    sh["b2c"] = np.asarray(b2, N_F32)[:, None].copy()
    sh["linbc"] = np.asarray(lin_b, N_F32)[:, None].copy()
    sh["gamc"] = np.asarray(bn_gamma, N_F32)[:, None].copy()
    sh["betc"] = np.asarray(bn_beta, N_F32)[:, None].copy()
    return sh


# ---------------------------------------------------------------------------
# Device program
# ---------------------------------------------------------------------------


def _build_program(m):
    import concourse.bass as bass
    import concourse.tile as tile
    from concourse import bacc, mybir

    f32 = mybir.dt.float32
    bf16 = mybir.dt.bfloat16
    i32 = mybir.dt.int32

    nc = bacc.Bacc(num_devices=m.C, num_swdge_queues=4)

    T = {}
    T["xs"] = nc.dram_tensor("xs", [m.NP, m.in_f], bf16, kind="ExternalInput")
    T["colsb"] = nc.dram_tensor("colsb", [128, m.CH], i32,
                                kind="ExternalInput")
    T["wsb"] = nc.dram_tensor("wsb", [128, m.CH], bf16, kind="ExternalInput")
    T["disl"] = nc.dram_tensor("disl", [128, m.NB], f32,
                               kind="ExternalInput")
    for k in range(3):
        T[f"w1_{k}"] = nc.dram_tensor(f"w1_{k}", [m.in_f, m.c1], bf16,
                                      kind="ExternalInput")
        T[f"w2_{k}"] = nc.dram_tensor(f"w2_{k}", [m.c1, m.c2], bf16,
                                      kind="ExternalInput")
    T["linwt"] = nc.dram_tensor("linwt", [m.c2, m.out_f], bf16,
                                kind="ExternalInput")
    for nm, p in (("b1c", m.c1), ("b2c", m.c2), ("linbc", m.out_f),
                  ("gamc", m.c1), ("betc", m.c1)):
        T[nm] = nc.dram_tensor(nm, [p, 1], f32, kind="ExternalInput")
    T["out"] = nc.dram_tensor("out", [m.NP, m.out_f], f32,
                              kind="ExternalOutput")

    with tile.TileContext(nc) as tc:
        _emit(nc, tc, m, T)
    nc.finalize()
    return nc


def _emit(nc, tc, m, T):
    from contextlib import ExitStack

    import concourse.bass as bass
    from concourse import mybir

    f32 = mybir.dt.float32
    bf16 = mybir.dt.bfloat16
    i32 = mybir.dt.int32
    OP = mybir.AluOpType
    ACT = mybir.ActivationFunctionType
    AX = mybir.AxisListType
    rg = [list(range(m.C))]
    NB, NP, RPC = m.NB, m.NP, m.RPC
    in_f, c1, c2, out_f = m.in_f, m.c1, m.c2, m.out_f
    CHUNK = 512
    nchunks = (NP + CHUNK - 1) // CHUNK

    with ExitStack() as ctx:
        cp = ctx.enter_context(tc.tile_pool(name="consts", bufs=1))
        gp = ctx.enter_context(tc.tile_pool(name="gth", bufs=2))
        bigf = ctx.enter_context(tc.tile_pool(name="bigf", bufs=1))
        nmp = ctx.enter_context(tc.tile_pool(name="nm", bufs=1))
        fmp = ctx.enter_context(tc.tile_pool(name="fm", bufs=1))
        rhp = ctx.enter_context(tc.tile_pool(name="rh", bufs=2))
        castp = ctx.enter_context(tc.tile_pool(name="cast", bufs=1))
        ep = ctx.enter_context(tc.tile_pool(name="epi", bufs=4))
        pp = ctx.enter_context(tc.tile_pool(name="ps", bufs=4, space="PSUM"))
        dp = ctx.enter_context(tc.tile_pool(name="dram", bufs=1,
                                            space="DRAM"))

        def load_const(name, shape, dtype):
            t = cp.tile(shape, dtype, tag=name, name=name)
            nc.sync.dma_start(out=t[:], in_=T[name][:])
            return t

        col_s = load_const("colsb", [128, m.CH], i32)
        w_s = load_const("wsb", [128, m.CH], bf16)
        disl = load_const("disl", [128, NB], f32)
        w1_s = [load_const(f"w1_{k}", [in_f, c1], bf16) for k in range(3)]
        w2_s = [load_const(f"w2_{k}", [c1, c2], bf16) for k in range(3)]
        linwt_s = load_const("linwt", [c2, out_f], bf16)
        b1c = load_const("b1c", [c1, 1], f32)
        b2c = load_const("b2c", [c2, 1], f32)
        linbc = load_const("linbc", [out_f, 1], f32)
        gamc = load_const("gamc", [c1, 1], f32)
        betc = load_const("betc", [c1, 1], f32)

        # doubled weights for the T2 = 2*L T1 - T0 step
        wx2 = cp.tile([128, m.CH], bf16, tag="wx2", name="wx2")
        nc.vector.tensor_scalar(out=wx2[:], in0=w_s[:], scalar1=2.0,
                                scalar2=None, op0=OP.mult)

        # node-major x (bf16) for the T2 subtraction
        x_sb = nmp.tile([128, NB * in_f], bf16, tag="x", name="x")
        nc.sync.dma_start(
            out=x_sb[:].rearrange("p (b f) -> p b f", b=NB),
            in_=T["xs"][:].rearrange("(b p) f -> p b f", p=128))

        stage = nmp.tile([128, NB * m.F], bf16, tag="stage", name="stage")

        # DRAM: shards, gathered tables, raw transposable scratch
        sh = [dp.tile([NP, in_f], bf16, tag="sh0", name="sh0"),
              dp.tile([NP, in_f], bf16, tag="sh1", name="sh1"),
              dp.tile([NP, c1], bf16, tag="sh2", name="sh2"),
              dp.tile([NP, c1], bf16, tag="sh3", name="sh3")]
        tb = [dp.tile([m.TN, in_f], bf16, tag="tb0", name="tb0",
                      addr_space="Shared"),
              dp.tile([m.TN, in_f], bf16, tag="tb1", name="tb1",
                      addr_space="Shared"),
              dp.tile([m.TN, c1], bf16, tag="tb2", name="tb2",
                      addr_space="Shared"),
              dp.tile([m.TN, c1], bf16, tag="tb3", name="tb3",
                      addr_space="Shared")]
        raw = {nm: dp.tile([NP, f], bf16, tag=nm, name=nm)
               for nm, f in (("t1r", in_f), ("t2r", in_f), ("hpr", c1),
                             ("t1pr", c1), ("t2pr", c1))}

        def stage_to_table(i, f):
            nc.sync.dma_start(
                out=sh[i][:].rearrange("(b p) f -> p b f", p=128),
                in_=stage[:, :NB * f].rearrange("p (b f) -> p b f", b=NB))
            nc.gpsimd.collective_compute(
                "AllGather", OP.bypass, replica_groups=rg,
                ins=[sh[i][:]], outs=[tb[i][:]])

        # table0 = disl * x
        nc.vector.tensor_tensor(
            out=stage[:, :NB * in_f].rearrange("p (b f) -> p b f", b=NB),
            in0=x_sb[:].rearrange("p (b f) -> p b f", b=NB),
            in1=disl[:].unsqueeze(2).broadcast_to([128, NB, in_f]),
            op=OP.mult)
        stage_to_table(0, in_f)

        # ------------ propagate: one gather/mul/reduce per group ----------
        # gather output is the NATURAL layout: slot (B, j)'s f features land
        # contiguously at column (B*Kg + j)*f — same per-index form as a
        # classic row gather. The weighted multiply and the slot-axis reduce
        # use strided 4D views of that buffer.
        qctr = [0]

        def propagate(table, f, weights, out_t):
            """out_t[:, b*f:(b+1)*f] = sum_j w[slot] * table[col[slot]]"""
            for gi, (b0, nblk, Kg) in enumerate(m.groups):
                goff = m.GOFF[gi]
                ncols = nblk * Kg
                g = gp.tile([128, m.GW], bf16, tag="g", name="g")
                # HW indirect DMA honors exactly ONE index per partition:
                # gather each slot column separately, round-robin the queues.
                for j in range(ncols):
                    inst = nc.gpsimd.indirect_dma_start(
                        out=g[:, j * f:(j + 1) * f], out_offset=None,
                        in_=table[:],
                        in_offset=bass.IndirectOffsetOnAxis(
                            ap=col_s[:, goff + j:goff + j + 1], axis=0))
                    qn = qctr[0] % 4
                    qctr[0] += 1
                    if qn:
                        inst.ins.queue = f"qPoolDynamic{qn}"
                gv = g[:, :ncols * f].rearrange("p (B j c) -> p B j c",
                                                B=nblk, c=f)
                nc.vector.tensor_tensor(
                    out=gv, in0=gv,
                    in1=weights[:, goff:goff + ncols]
                        .rearrange("p (B j) -> p B j", B=nblk)
                        .unsqueeze(3).broadcast_to([128, nblk, Kg, f]),
                    op=OP.mult)
                nc.vector.tensor_reduce(
                    out=out_t[:, b0 * f:(b0 + nblk) * f].rearrange(
                        "p (B c) -> p B c", B=nblk),
                    in_=g[:, :ncols * f].rearrange("p (B j c) -> p B c j",
                                                   B=nblk, c=f),
                    axis=AX.X, op=OP.add)

        # ------------ conv1 propagates ------------
        T1 = bigf.tile([128, NB * m.F], f32, tag="bigf", name="T1")
        propagate(tb[0][:], in_f, w_s, T1)
        nc.vector.tensor_tensor(
            out=stage[:, :NB * in_f].rearrange("p (b f) -> p b f", b=NB),
            in0=T1[:, :NB * in_f].rearrange("p (b f) -> p b f", b=NB),
            in1=disl[:].unsqueeze(2).broadcast_to([128, NB, in_f]),
            op=OP.mult)
        stage_to_table(1, in_f)
        # T1 -> bf16 -> DRAM (for feature-major dense reads)
        t1bf = castp.tile([128, NB * in_f], bf16, tag="cast", name="cast")
        nc.scalar.copy(out=t1bf[:], in_=T1[:, :NB * in_f])
        nc.sync.dma_start(
            out=raw["t1r"][:].rearrange("(b p) f -> p b f", p=128),
            in_=t1bf[:].rearrange("p (b f) -> p b f", b=NB))

        R = bigf.tile([128, NB * m.F], f32, tag="bigf", name="R")
        propagate(tb[1][:], in_f, wx2, R)
        t2bf = castp.tile([128, NB * in_f], bf16, tag="cast", name="cast")
        nc.vector.tensor_tensor(out=t2bf[:], in0=R[:, :NB * in_f],
                                in1=x_sb[:], op=OP.subtract)
        nc.sync.dma_start(
            out=raw["t2r"][:].rearrange("(b p) f -> p b f", p=128),
            in_=t2bf[:].rearrange("p (b f) -> p b f", b=NB))

        # ------------ conv1 dense (feature-major, chunked) ------------
        # hT[c, n] = relu(sum_k W1_k^T @ Tk^T + b1)
        hT = fmp.tile([c1, NP], bf16, tag="hT", name="hT")
        LG = 4 * CHUNK                      # rhs load-group width (one DMA)
        nlg = (NP + LG - 1) // LG

        def dense(ws, srcs_dram, fin, fout, bias, sink, extra_rhs=None):
            """sink(ci, n0, n1, psum_ap) consumes relu-less psum chunks.

            srcs_dram[k] is a node-major [NP, fin] DRAM tensor read
            feature-major in LG-wide load groups; extra_rhs (SBUF,
            feature-major) overrides source k=0.
            """
            for li in range(nlg):
                l0 = li * LG
                l1 = min(NP, l0 + LG)
                rts = []
                for k in range(3):
                    if extra_rhs is not None and k == 0:
                        rts.append(None)
                        continue
                    r = rhp.tile([fin, LG], bf16, tag=f"rh{k}",
                                 name=f"rh{k}")
                    nc.sync.dma_start(
                        out=r[:, :l1 - l0],
                        in_=srcs_dram[k][l0:l1, :].rearrange("n f -> f n"))
                    rts.append(r)
                for ci in range((l1 - l0 + CHUNK - 1) // CHUNK):
                    n0 = l0 + ci * CHUNK
                    n1 = min(l1, n0 + CHUNK)
                    ps = pp.tile([fout, CHUNK], f32, tag="mm", name="mm")
                    for k in range(3):
                        if extra_rhs is not None and k == 0:
                            rhs = extra_rhs[:, n0:n1]
                        else:
                            rhs = rts[k][:, n0 - l0:n1 - l0]
                        nc.tensor.matmul(out=ps[:, :n1 - n0], lhsT=ws[k][:],
                                         rhs=rhs, start=(k == 0),
                                         stop=(k == 2))
                    sink(n0, n1, ps)

        def h_sink(n0, n1, ps):
            nc.scalar.activation(out=hT[:, n0:n1], in_=ps[:, :n1 - n0],
                                 func=ACT.Relu, bias=b1c[:], scale=1.0)

        dense(w1_s, [T["xs"], raw["t1r"], raw["t2r"]], in_f, c1, b1c, h_sink)

        # zero the padded node columns so BN stats see only real rows
        if RPC < NP:
            nc.vector.memset(hT[:, RPC:NP], 0.0)

        # ------------ BatchNorm (feature-major) ------------
        s12 = cp.tile([c1, 2], f32, tag="s12", name="s12")
        nc.vector.tensor_reduce(out=s12[:, 0:1], in_=hT[:], axis=AX.X,
                                op=OP.add)
        # E[h^2]: chunked square with per-chunk accumulators, then reduce
        s2c = cp.tile([c1, nchunks], f32, tag="s2c", name="s2c")
        for ci in range(nchunks):
            n0 = ci * CHUNK
            n1 = min(NP, n0 + CHUNK)
            sq = ep.tile([c1, CHUNK], bf16, tag="sq", name="sq")
            nc.scalar.activation(out=sq[:, :n1 - n0], in_=hT[:, n0:n1],
                                 func=ACT.Square,
                                 accum_out=s2c[:, ci:ci + 1])
        nc.vector.tensor_reduce(out=s12[:, 1:2], in_=s2c[:], axis=AX.X,
                                op=OP.add)
        st_l = dp.tile([c1, 2], f32, tag="st_l", name="st_l")
        st_g = dp.tile([c1, 2], f32, tag="st_g", name="st_g",
                       addr_space="Shared")
        nc.sync.dma_start(out=st_l[:], in_=s12[:])
        nc.gpsimd.collective_compute("AllReduce", OP.add, replica_groups=rg,
                                     ins=[st_l[:]], outs=[st_g[:]])
        gst = cp.tile([c1, 2], f32, tag="gst", name="gst")
        nc.sync.dma_start(out=gst[:], in_=st_g[:])

        def colv(tag):
            return cp.tile([c1, 1], f32, tag=tag, name=tag)

        mu, ex2, var, vrec, vrs, gp_, bp_ = (colv(t) for t in
            ("mu", "ex2", "var", "vrec", "vrs", "gp", "bp"))
        inv_n = 1.0 / float(m.N)
        nc.vector.tensor_scalar(out=mu[:], in0=gst[:, 0:1], scalar1=inv_n,
                                scalar2=None, op0=OP.mult)
        nc.vector.tensor_scalar(out=ex2[:], in0=gst[:, 1:2], scalar1=inv_n,
                                scalar2=None, op0=OP.mult)
        nc.vector.tensor_tensor(out=var[:], in0=mu[:], in1=mu[:], op=OP.mult)
        nc.vector.tensor_tensor(out=var[:], in0=ex2[:], in1=var[:],
                                op=OP.subtract)
        nc.vector.tensor_scalar(out=var[:], in0=var[:], scalar1=1e-5,
                                scalar2=None, op0=OP.add)
        nc.vector.reciprocal(out=vrec[:], in_=var[:])
        nc.scalar.sqrt(out=vrs[:], in_=vrec[:])
        nc.vector.tensor_tensor(out=gp_[:], in0=gamc[:], in1=vrs[:],
                                op=OP.mult)
        nc.vector.tensor_tensor(out=bp_[:], in0=mu[:], in1=gp_[:],
                                op=OP.mult)
        nc.vector.tensor_tensor(out=bp_[:], in0=betc[:], in1=bp_[:],
                                op=OP.subtract)
        # h' = g'*h + b'  (in place, feature-major)
        nc.vector.tensor_scalar(out=hT[:], in0=hT[:], scalar1=gp_[:],
                                scalar2=bp_[:], op0=OP.mult, op1=OP.add)

        # h' node-major (for table + T2p subtraction) via DRAM round trip
        nc.sync.dma_start(out=raw["hpr"][:].rearrange("n f -> f n"),
                          in_=hT[:])
        hp_sb = nmp.tile([128, NB * c1], bf16, tag="hp", name="hp")
        nc.sync.dma_start(
            out=hp_sb[:].rearrange("p (b f) -> p b f", b=NB),
            in_=raw["hpr"][:].rearrange("(b p) f -> p b f", p=128))
        nc.vector.tensor_tensor(
            out=stage[:, :NB * c1].rearrange("p (b f) -> p b f", b=NB),
            in0=hp_sb[:].rearrange("p (b f) -> p b f", b=NB),
            in1=disl[:].unsqueeze(2).broadcast_to([128, NB, c1]),
            op=OP.mult)
        stage_to_table(2, c1)

        # ------------ conv2 propagates ------------
        T1p = bigf.tile([128, NB * m.F], f32, tag="bigf", name="T1p")
        propagate(tb[2][:], c1, w_s, T1p)
        nc.vector.tensor_tensor(
            out=stage[:, :NB * c1].rearrange("p (b f) -> p b f", b=NB),
            in0=T1p[:, :NB * c1].rearrange("p (b f) -> p b f", b=NB),
            in1=disl[:].unsqueeze(2).broadcast_to([128, NB, c1]),
            op=OP.mult)
        stage_to_table(3, c1)
        t1pbf = castp.tile([128, NB * c1], bf16, tag="cast", name="cast")
        nc.scalar.copy(out=t1pbf[:], in_=T1p[:, :NB * c1])
        nc.sync.dma_start(
            out=raw["t1pr"][:].rearrange("(b p) f -> p b f", p=128),
            in_=t1pbf[:].rearrange("p (b f) -> p b f", b=NB))

        R2 = bigf.tile([128, NB * m.F], f32, tag="bigf", name="R2")
        propagate(tb[3][:], c1, wx2, R2)
        t2pbf = castp.tile([128, NB * c1], bf16, tag="cast", name="cast")
        nc.vector.tensor_tensor(out=t2pbf[:], in0=R2[:, :NB * c1],
                                in1=hp_sb[:], op=OP.subtract)
        nc.sync.dma_start(
            out=raw["t2pr"][:].rearrange("(b p) f -> p b f", p=128),
            in_=t2pbf[:].rearrange("p (b f) -> p b f", b=NB))

        # ------------ conv2 dense + fused final linear ------------
        def h2_sink(n0, n1, ps):
            h2c = ep.tile([c2, CHUNK], bf16, tag="h2c", name="h2c")
            nc.scalar.activation(out=h2c[:, :n1 - n0], in_=ps[:, :n1 - n0],
                                 func=ACT.Relu, bias=b2c[:], scale=1.0)
            pl = pp.tile([out_f, CHUNK], f32, tag="lin", name="lin")
            nc.tensor.matmul(out=pl[:, :n1 - n0], lhsT=linwt_s[:],
                             rhs=h2c[:, :n1 - n0], start=True, stop=True)
            ot = ep.tile([out_f, CHUNK], f32, tag="ot", name="ot")
            nc.vector.tensor_scalar(out=ot[:, :n1 - n0], in0=pl[:, :n1 - n0],
                                    scalar1=linbc[:], scalar2=None,
                                    op0=OP.add)
            nc.sync.dma_start(
                out=T["out"][n0:n1, :].rearrange("n f -> f n"),
                in_=ot[:, :n1 - n0])

        dense(w2_s, [None, raw["t1pr"], raw["t2pr"]], c1, c2, b2c, h2_sink,
              extra_rhs=hT)


# ---------------------------------------------------------------------------
# Entry point
# ---------------------------------------------------------------------------


def _run(inputs, n_cores=8, trace=False):
    from concourse.bass_utils import run_bass_kernel_spmd

    m, in_maps = _host_prep(n_cores=n_cores, **inputs)
    nc = _build_program(m)
    res = run_bass_kernel_spmd(nc, in_maps, core_ids=list(range(n_cores)),
                               trace=trace)
    out = np.empty((m.N, m.out_f), dtype=N_F32)
    for c in range(m.C):
        lo = c * m.RPC
        out[lo:lo + m.RPC] = np.asarray(
            res.results[c]["out"], dtype=N_F32)[m.pos[lo:lo + m.RPC]]
    return out, res


def kernel(**inputs):
    out, _ = _run(inputs, n_cores=8, trace=False)
    return out
